# revision 28
# baseline (speedup 1.0000x reference)
"""Trainium2 Bass kernel for nn_Attention_91087666414100 (sparse/divided
space-time attention block, TimeSformer-style spatial attention with a cls
token).

Strategy: data-parallel over batch B=8 -> one batch element per NeuronCore.
Each core computes the full module for its batch:
  qkv = x @ W_qkv ; per-head scaled-dot-product attention where the cls token
  attends over everything and each frame's 196 spatial tokens attend over
  (cls + own frame) ; out = attn @ W_out + b_out.

On-chip dataflow is kept "d-major" (transposed): qT/kT live as [hd, n] so all
matmuls contract over the partition dim with no transposes in the hot loop.
fp16 operands, fp32 PSUM accumulation. Softmax uses exp without max
subtraction (logits are O(5), safe in fp32) with the 1/8 scale folded into
the ScalarE exp; the denominator comes for free from a ones-column appended
to v; normalization is applied via a ones (x) (1/s) PE outer-product with a
bf16 hi/lo two-pass for near-fp32 precision on the reciprocal.
"""

import sys

if "/opt/trn_rl_repo" not in sys.path:
    sys.path.insert(0, "/opt/trn_rl_repo")

import numpy as np

import concourse.bass as bass
import concourse.tile as tile
from concourse import bacc, mybir
from concourse.masks import make_identity

F16 = mybir.dt.float16
F32 = mybir.dt.float32
BF16 = mybir.dt.bfloat16
EXPF = mybir.ActivationFunctionType.Exp

B = 8
NTOT = 1569        # 1 + F*N
DIM = 768
H = 12             # heads
D = 64             # dim per head
G = 6              # 128-partition groups of (h d): 2 heads per group
NF = 8             # frames
N = 196            # spatial tokens per frame
NSPA = NTOT - 1    # 1568
SCALE = D ** -0.5  # 0.125
C3 = 3 * H * D     # 2304


def _nchunks(total, step):
    return [(c, min(step, total - c)) for c in range(0, total, step)]


def build_bass() -> bass.Bass:
    # Bacc (not raw Bass): its compile pipeline runs generate_event_semaphores,
    # which legalizes instructions to <=1 sync wait (the DGE DMA limit).
    nc = bacc.Bacc()

    x_d = nc.declare_dram_parameter("x", [NTOT, DIM], F32, isOutput=False)
    wqkv_d = nc.declare_dram_parameter("W_qkv", [DIM, C3], F32, isOutput=False)
    wout_d = nc.declare_dram_parameter("W_out", [DIM, DIM], F32, isOutput=False)
    bout_d = nc.declare_dram_parameter("b_out", [DIM], F32, isOutput=False)
    out_d = nc.declare_dram_parameter("out", [NTOT, DIM], F32, isOutput=True)
    clsa_d = nc.declare_dram_parameter("cls_attn", [H, NTOT], F32, isOutput=True)

    with tile.TileContext(nc) as tc:
        _emit(nc, tc, x_d, wqkv_d, wout_d, bout_d, out_d, clsa_d)
    if not nc.is_finalized():
        nc.finalize()
    return nc


def _emit(nc, tc, x_d, wqkv_d, wout_d, bout_d, out_d, clsa_d):
    from contextlib import ExitStack

    ctx = ExitStack()
    with ctx:
        persist = ctx.enter_context(tc.tile_pool(name="persist", bufs=1))

        # ---- persistent tiles ----
        wo16 = persist.tile([128, G, DIM], F16)          # W_out, row-blocks
        qT = persist.tile([128, G, NTOT], F16)           # q^T  [hd, n]
        kT = persist.tile([128, G, NTOT], F16)           # k^T  [hd, n]
        # v natural, frame-padded, 65-col interleave: [.., h-block] =
        # [v_h (64 cols) | ones (1 col)]
        v_sb = persist.tile([128, NF, 2, H * 65], F16)
        cls_v65 = persist.tile([1, H * 65], F16)
        idf16 = persist.tile([128, 128], F16)            # identity for PE transpose
        ones16 = persist.tile([1, 128], F16)
        onesbf = persist.tile([1, 64], BF16)
        b16 = persist.tile([1, DIM], F16)
        co16 = persist.tile([H, DIM], F16)               # cls-token attn output

        make_identity(nc, idf16)
        nc.vector.memset(ones16, 1.0)
        nc.vector.memset(onesbf, 1.0)

        bstage = persist.tile([1, DIM], F32)
        nc.gpsimd.dma_start(out=bstage, in_=bout_d[None, :])
        nc.vector.tensor_copy(out=b16, in_=bstage)

        # ones columns of the v'' layout
        for t in range(NF):
            for sub in range(2):
                v3 = v_sb[:, t, sub, :].rearrange("p (h w) -> p h w", w=65)
                nc.vector.memset(v3[:, :, 64:65], 1.0)
        cv3 = cls_v65[:, :].rearrange("p (h w) -> p h w", w=65)
        nc.vector.memset(cv3[:, :, 64:65], 1.0)

        # ---------------- P1..P3: load weights/x, build xT, qT/kT, v ---------
        # Every dma_start writes a one-shot disjoint region: the hardware
        # dynamic-DGE DMA instruction supports only ONE semaphore wait, and
        # pool-slot reuse makes Tile attach two (reader-release + cross-queue
        # WAW), which walrus rejects ("Too many sync wait commands").
        with tc.tile_pool(name="p13", bufs=1) as p13, \
             tc.tile_pool(name="pp1", bufs=2, space="PSUM") as pp1, \
             tc.tile_pool(name="pp2", bufs=4, space="PSUM") as pp2:

            w16 = p13.tile([128, G, C3], F16)            # W_qkv row-blocks
            xT = p13.tile([128, G, NTOT], F16)           # x^T   [dim, n]

            with tc.tile_pool(name="wstage", bufs=1) as wstage:
                w32 = wstage.tile([128, G, C3], F32)
                for k in range(G):
                    nc.sync.dma_start(
                        out=w32[:, k, :], in_=wqkv_d[128 * k:128 * (k + 1), :]
                    )
                    nc.vector.tensor_copy(out=w16[:, k, :], in_=w32[:, k, :])

            with tc.tile_pool(name="wstage2", bufs=1) as wstage2:
                wo32 = wstage2.tile([128, G, DIM], F32)
                # memset-before-DMA: the pool region is recycled, and the DMA
                # would otherwise inherit one release-wait per prior accessor
                # proc (hardware DMA allows a single sync wait). The memset
                # (DVE, multi-wait capable) absorbs them all.
                nc.vector.memset(wo32, 0.0)
                for k in range(G):
                    nc.sync.dma_start(
                        out=wo32[:, k, :], in_=wout_d[128 * k:128 * (k + 1), :]
                    )
                    nc.vector.tensor_copy(out=wo16[:, k, :], in_=wo32[:, k, :])

            # x -> fp16 -> PE-transpose into xT
            with tc.tile_pool(name="xstage", bufs=1) as xstage:
                x32 = xstage.tile([128, 13, DIM], F32)
                x16 = xstage.tile([128, 13, DIM], F16)
                nc.vector.memset(x32, 0.0)
                for nt in range(13):
                    r0 = 128 * nt
                    m = min(128, NTOT - r0)
                    nc.sync.dma_start(
                        out=x32[:m, nt, :], in_=x_d[r0:r0 + m, :]
                    )
                    nc.vector.tensor_copy(out=x16[:m, nt, :], in_=x32[:m, nt, :])
                    for db in range(G):
                        pt = pp1.tile([128, 128], F16, tag="pt")
                        nc.tensor.transpose(
                            pt[0:128, 0:m],
                            x16[0:m, nt, 128 * db:128 * (db + 1)],
                            idf16[0:m, 0:m],
                        )
                        nc.vector.tensor_copy(
                            out=xT[:, db, r0:r0 + m], in_=pt[0:128, 0:m]
                        )

            # ---- P2: qT / kT projection (c-groups 0..11 of qkv^T) ----
            for ct in range(12):
                dst = qT if ct < G else kT
                gi = ct % G
                for cs, cw in _nchunks(NTOT, 512):
                    pq = pp2.tile([128, 512], F32, tag="mm")
                    for k in range(G):
                        nc.tensor.matmul(
                            pq[:, 0:cw],
                            w16[:, k, 128 * ct:128 * (ct + 1)],
                            xT[:, k, cs:cs + cw],
                            start=(k == 0),
                            stop=(k == G - 1),
                        )
                    nc.vector.tensor_copy(out=dst[:, gi, cs:cs + cw], in_=pq[:, 0:cw])

            # ---- P3: v natural (frame-padded, 65-interleaved) ----
            def v_proj(row0, m, dst_ap):
                # dst_ap: [m, 12, 64]-style destination (65-strided blocks)
                for ch in range(2):
                    pv = pp2.tile([128, 512], F32, tag="mm", name="pv")
                    for k in range(G):
                        nc.tensor.matmul(
                            pv[0:m, 0:384],
                            xT[:, k, row0:row0 + m],
                            w16[:, k, 2 * DIM + 384 * ch:2 * DIM + 384 * (ch + 1)],
                            start=(k == 0),
                            stop=(k == G - 1),
                        )
                    src3 = pv[0:m, 0:384].rearrange("p (h w) -> p h w", w=64)
                    nc.vector.tensor_copy(
                        out=dst_ap[:, 6 * ch:6 * (ch + 1), 0:64], in_=src3
                    )

            v_proj(0, 1, cls_v65[0:1, :].rearrange("p (h w) -> p h w", w=65))
            # frame j-split {96, 100}: the cls key/value rides at row 96 of
            # sub0 (96 is a legal engine partition base; 68 is not)
            for t in range(NF):
                for sub in range(2):
                    m = 96 if sub == 0 else 100
                    row0 = 1 + N * t + 96 * sub
                    dst3 = v_sb[0:m, t, sub, :].rearrange("p (h w) -> p h w", w=65)
                    v_proj(row0, m, dst3)
            for t in range(NF):
                nc.gpsimd.tensor_copy(
                    out=v_sb[96:97, t, 0, :], in_=cls_v65[0:1, :]
                )

        # ---------------- P4: cls-as-query attention row ----------------------
        p45 = ctx.enter_context(tc.tile_pool(name="p45", bufs=1))
        attnT = p45.tile([128, G, NTOT], F16)   # attention out^T
        e12 = p45.tile([H, NTOT], F16)          # exp(cls-query sims)
        a12 = p45.tile([H, NTOT], F32)          # normalized cls attention (output)
        a16 = p45.tile([H, NTOT], F16)
        s_cls = p45.tile([H, 1], F32)
        rs_cls = p45.tile([H, 1], F32)
        # selector weights: cq12[:, g, 2g+p] = cls_q for head 2g+p, else 0 ->
        # the 6 per-group matmuls accumulate into one [12, n] psum
        cq12 = p45.tile([128, G, H], F16)

        with tc.tile_pool(name="pp4", bufs=4, space="PSUM") as pp4:
            nc.vector.memset(cq12, 0.0)
            for g in range(G):
                nc.gpsimd.tensor_copy(
                    out=cq12[0:64, g, 2 * g:2 * g + 1], in_=qT[0:64, g, 0:1]
                )
                nc.gpsimd.tensor_copy(
                    out=cq12[64:128, g, 2 * g + 1:2 * g + 2], in_=qT[64:128, g, 0:1]
                )
            for cs, cw in _nchunks(NTOT, 512):
                pA = pp4.tile([H, 512], F32, tag="pA")
                for g in range(G):
                    nc.tensor.matmul(
                        pA[:, 0:cw], cq12[:, g, :], kT[:, g, cs:cs + cw],
                        start=(g == 0), stop=(g == G - 1),
                    )
                nc.scalar.activation(
                    out=e12[:, cs:cs + cw], in_=pA[:, 0:cw], func=EXPF, scale=SCALE
                )

        nc.vector.reduce_sum(out=s_cls, in_=e12, axis=mybir.AxisListType.X)
        nc.vector.reciprocal(out=rs_cls, in_=s_cls)
        # a12 = e12 * rs (per-partition scale) with f16->f32 cast, on ScalarE
        nc.scalar.activation(
            out=a12, in_=e12, func=mybir.ActivationFunctionType.Copy, scale=rs_cls
        )
        nc.sync.dma_start(out=clsa_d[:, :], in_=a12)
        nc.vector.tensor_copy(out=a16, in_=a12)

        # ---------------- P5: frame attention (96 head x frame tiles) --------
        with tc.tile_pool(name="p5t", bufs=3) as p5t, \
             tc.tile_pool(name="pp5", bufs=2, space="PSUM") as pp5:
            for h in range(H):
                g, po = h // 2, 64 * (h % 2)
                for t in range(NF):
                    qs = 1 + N * t
                    ps = pp5.tile([128, 392], F32, tag="ps")
                    # j-tile0: frame keys 0:96 (+ cls-key sim row at part 96)
                    nc.tensor.matmul(
                        ps[0:96, 0:N],
                        kT[po:po + 64, g, qs:qs + 96],
                        qT[po:po + 64, g, qs:qs + N],
                        start=True, stop=True,
                    )
                    ps_cls = pp5.tile([1, N], F32, tag="ps_cls")
                    nc.tensor.matmul(
                        ps_cls,
                        kT[po:po + 64, g, 0:1],
                        qT[po:po + 64, g, qs:qs + N],
                        start=True, stop=True,
                    )
                    # j-tile1: frame keys 96:196
                    nc.tensor.matmul(
                        ps[0:100, N:2 * N],
                        kT[po:po + 64, g, qs + 96:qs + N],
                        qT[po:po + 64, g, qs:qs + N],
                        start=True, stop=True,
                    )
                    eT0 = p5t.tile([97, N], F16, tag="eT0")
                    eT1 = p5t.tile([100, N], F16, tag="eT1")
                    nc.scalar.activation(
                        out=eT0[0:96, :], in_=ps[0:96, 0:N], func=EXPF, scale=SCALE
                    )
                    nc.scalar.activation(
                        out=eT0[96:97, :], in_=ps_cls, func=EXPF, scale=SCALE
                    )
                    nc.scalar.activation(
                        out=eT1, in_=ps[0:100, N:2 * N], func=EXPF, scale=SCALE
                    )
                    po_t = pp5.tile([65, N], F32, tag="po")
                    nc.tensor.matmul(
                        po_t, v_sb[0:97, t, 0, 65 * h:65 * h + 65], eT0,
                        start=True, stop=False,
                    )
                    nc.tensor.matmul(
                        po_t, v_sb[0:100, t, 1, 65 * h:65 * h + 65], eT1,
                        start=False, stop=True,
                    )
                    # normalize: rs = 1/s ; attnT = out2T * (ones (x) rs)
                    rsf = p5t.tile([1, N], F32, tag="rsf")
                    rhi = p5t.tile([1, N], BF16, tag="rhi")
                    rhi32 = p5t.tile([1, N], F32, tag="rhi32")
                    rlo = p5t.tile([1, N], BF16, tag="rlo")
                    nc.vector.reciprocal(out=rsf, in_=po_t[64:65, :])
                    nc.vector.tensor_copy(out=rhi, in_=rsf)
                    nc.vector.tensor_copy(out=rhi32, in_=rhi)
                    nc.vector.tensor_sub(rlo, rsf, rhi32)
                    pM = pp5.tile([64, N], F32, tag="pM")
                    nc.tensor.matmul(pM, onesbf, rhi, start=True, stop=False)
                    nc.tensor.matmul(pM, onesbf, rlo, start=False, stop=True)
                    o32 = p5t.tile([64, N], F32, tag="o32")
                    nc.vector.tensor_copy(out=o32, in_=po_t[0:64, :])
                    nc.vector.tensor_mul(
                        attnT[po:po + 64, g, qs:qs + N], o32, pM
                    )

        # ---------------- P7: cls-token output -> attnT column 0 -------------
        with tc.tile_pool(name="p7t", bufs=3) as p7t, \
             tc.tile_pool(name="pp7", bufs=2, space="PSUM") as pp7a, \
             tc.tile_pool(name="pp7s", bufs=3, space="PSUM") as pp7s:
            co_ps = [
                pp7a.tile([H, 384], F32, tag="pco", name=f"pco{i}") for i in range(2)
            ]
            # seg list: (transposed a_cls slice, matching v rows)
            segs = []
            pt0 = pp7s.tile([128, H], F16, tag="ptseg")
            nc.tensor.transpose(pt0[0:1, 0:H], a16[0:H, 0:1], idf16[0:H, 0:H])
            a0 = p7t.tile([1, H], F16, tag="aseg")
            nc.vector.tensor_copy(out=a0, in_=pt0[0:1, 0:H])
            segs.append((a0, 1, cls_v65[0:1, :].rearrange("p (h w) -> p h w", w=65)))
            for t in range(NF):
                for sub in range(2):
                    kk = 96 if sub == 0 else 100
                    c0 = 1 + N * t + 96 * sub
                    ptk = pp7s.tile([128, H], F16, tag="ptseg")
                    nc.tensor.transpose(
                        ptk[0:kk, 0:H], a16[0:H, c0:c0 + kk], idf16[0:H, 0:H]
                    )
                    ak = p7t.tile([128, H], F16, tag="aseg")
                    nc.vector.tensor_copy(out=ak[0:kk, :], in_=ptk[0:kk, 0:H])
                    v3 = v_sb[0:kk, t, sub, :].rearrange("p (h w) -> p h w", w=65)
                    segs.append((ak, kk, v3))
            for ch in range(2):
                for i, (aseg, kk, v3) in enumerate(segs):
                    nc.tensor.matmul(
                        co_ps[ch],
                        aseg[0:kk, 0:H],
                        v3[:, 6 * ch:6 * (ch + 1), 0:64],
                        start=(i == 0),
                        stop=(i == len(segs) - 1),
                    )
            for ch in range(2):
                nc.vector.tensor_copy(
                    out=co16[:, 384 * ch:384 * (ch + 1)], in_=co_ps[ch]
                )
            for g in range(G):
                ptc = pp7s.tile([128, H], F16, tag="ptc")
                nc.tensor.transpose(
                    ptc[0:128, 0:H], co16[0:H, 128 * g:128 * (g + 1)],
                    idf16[0:H, 0:H],
                )
                nc.vector.tensor_copy(
                    out=attnT[0:64, g, 0:1], in_=ptc[0:64, 2 * g:2 * g + 1]
                )
                nc.vector.tensor_copy(
                    out=attnT[64:128, g, 0:1], in_=ptc[64:128, 2 * g + 1:2 * g + 2]
                )

        # ---------------- P8: output projection + bias ------------------------
        with tc.tile_pool(name="p8t", bufs=3) as p8t, \
             tc.tile_pool(name="pp8", bufs=4, space="PSUM") as pp8:
            for nt in range(13):
                r0 = 128 * nt
                m = min(128, NTOT - r0)
                osb = p8t.tile([128, DIM], F32, tag="osb")
                for ch in range(2):
                    pf = pp8.tile([128, 384], F32, tag="pf")
                    for g in range(G):
                        nc.tensor.matmul(
                            pf[0:m, :],
                            attnT[:, g, r0:r0 + m],
                            wo16[:, g, 384 * ch:384 * (ch + 1)],
                            start=(g == 0),
                            stop=False,
                        )
                    nc.tensor.matmul(
                        pf[0:m, :],
                        ones16[0:1, 0:m],
                        b16[0:1, 384 * ch:384 * (ch + 1)],
                        start=False,
                        stop=True,
                    )
                    nc.vector.tensor_copy(
                        out=osb[0:m, 384 * ch:384 * (ch + 1)], in_=pf[0:m, :]
                    )
                nc.sync.dma_start(out=out_d[r0:r0 + m, :], in_=osb[0:m, :])


_NC_CACHE = None


def _get_nc():
    global _NC_CACHE
    if _NC_CACHE is None:
        _NC_CACHE = build_bass()
    return _NC_CACHE


LAST_RESULTS = None


def kernel(x, W_qkv, W_out, b_out, f):
    import os

    from concourse.bass_utils import run_bass_kernel_spmd

    global LAST_RESULTS
    x = np.ascontiguousarray(np.asarray(x), dtype=np.float32)
    W_qkv = np.ascontiguousarray(np.asarray(W_qkv), dtype=np.float32)
    W_out = np.ascontiguousarray(np.asarray(W_out), dtype=np.float32)
    b_out = np.ascontiguousarray(np.asarray(b_out), dtype=np.float32)
    assert int(f) == NF and x.shape == (B, NTOT, DIM)

    nc = _get_nc()
    in_maps = [
        {"x": x[b], "W_qkv": W_qkv, "W_out": W_out, "b_out": b_out}
        for b in range(B)
    ]
    res = run_bass_kernel_spmd(
        nc, in_maps, list(range(B)),
        trace=bool(os.environ.get("KBENCH_TRACE")),
    )
    LAST_RESULTS = res
    out = np.stack([res.results[b]["out"] for b in range(B)]).astype(np.float32)
    cls_attn = np.stack(
        [res.results[b]["cls_attn"] for b in range(B)]
    ).reshape(B * H, 1, NTOT).astype(np.float32)
    return out, cls_attn


# revision 29
# speedup vs baseline: 162.5894x; 162.5894x over previous
"""Trainium2 Bass kernel for nn_Attention_91087666414100 (sparse/divided
space-time attention block, TimeSformer-style spatial attention with a cls
token).

Strategy: data-parallel over batch B=8 -> one batch element per NeuronCore.
Each core computes the full module for its batch:
  qkv = x @ W_qkv ; per-head scaled-dot-product attention where the cls token
  attends over everything and each frame's 196 spatial tokens attend over
  (cls + own frame) ; out = attn @ W_out + b_out.

On-chip dataflow is kept "d-major" (transposed): qT/kT live as [hd, n] so all
matmuls contract over the partition dim with no transposes in the hot loop.
fp16 operands, fp32 PSUM accumulation. Softmax uses exp without max
subtraction (logits are O(5), safe in fp32) with the 1/8 scale folded into
the ScalarE exp; the denominator comes for free from a ones-column appended
to v; normalization is applied via a ones (x) (1/s) PE outer-product with a
bf16 hi/lo two-pass for near-fp32 precision on the reciprocal.
"""

import sys

if "/opt/trn_rl_repo" not in sys.path:
    sys.path.insert(0, "/opt/trn_rl_repo")

import numpy as np

import concourse.bass as bass
import concourse.tile as tile
from concourse import bacc, mybir
from concourse.masks import make_identity

F16 = mybir.dt.float16
F32 = mybir.dt.float32
BF16 = mybir.dt.bfloat16
EXPF = mybir.ActivationFunctionType.Exp

B = 8
NTOT = 1569        # 1 + F*N
DIM = 768
H = 12             # heads
D = 64             # dim per head
G = 6              # 128-partition groups of (h d): 2 heads per group
NF = 8             # frames
N = 196            # spatial tokens per frame
NSPA = NTOT - 1    # 1568
SCALE = D ** -0.5  # 0.125
C3 = 3 * H * D     # 2304


def _nchunks(total, step):
    return [(c, min(step, total - c)) for c in range(0, total, step)]


def build_bass(passes: int = 1) -> bass.Bass:
    # Bacc (not raw Bass): its compile pipeline runs generate_event_semaphores,
    # which legalizes instructions to <=1 sync wait (the DGE DMA limit).
    nc = bacc.Bacc()

    x_d = nc.declare_dram_parameter("x", [NTOT, DIM], F32, isOutput=False)
    wqkv_d = nc.declare_dram_parameter("W_qkv", [DIM, C3], F32, isOutput=False)
    wout_d = nc.declare_dram_parameter("W_out", [DIM, DIM], F32, isOutput=False)
    bout_d = nc.declare_dram_parameter("b_out", [DIM], F32, isOutput=False)
    out_d = nc.declare_dram_parameter("out", [NTOT, DIM], F32, isOutput=True)
    clsa_d = nc.declare_dram_parameter("cls_attn", [H, NTOT], F32, isOutput=True)

    with tile.TileContext(nc) as tc:
        for _ in range(passes):
            _emit(nc, tc, x_d, wqkv_d, wout_d, bout_d, out_d, clsa_d)
    if not nc.is_finalized():
        nc.finalize()
    return nc


def _emit(nc, tc, x_d, wqkv_d, wout_d, bout_d, out_d, clsa_d):
    from contextlib import ExitStack

    ctx = ExitStack()
    with ctx:
        persist = ctx.enter_context(tc.tile_pool(name="persist", bufs=1))

        # ---- persistent tiles ----
        wo16 = persist.tile([128, G, DIM], F16)          # W_out, row-blocks
        qT = persist.tile([128, G, NTOT], F16)           # q^T  [hd, n]
        kT = persist.tile([128, G, NTOT], F16)           # k^T  [hd, n]
        # v natural, frame-padded, 65-col interleave: [.., h-block] =
        # [v_h (64 cols) | ones (1 col)]
        v_sb = persist.tile([128, NF, 2, H * 65], F16)
        cls_v65 = persist.tile([1, H * 65], F16)
        idf16 = persist.tile([128, 128], F16)            # identity for PE transpose
        ones16 = persist.tile([1, 128], F16)
        onesbf = persist.tile([1, 64], BF16)
        b16 = persist.tile([1, DIM], F16)
        co16 = persist.tile([H, DIM], F16)               # cls-token attn output

        make_identity(nc, idf16)
        nc.vector.memset(ones16, 1.0)
        nc.vector.memset(onesbf, 1.0)

        bstage = persist.tile([1, DIM], F32)
        nc.gpsimd.dma_start(out=bstage, in_=bout_d[None, :])
        nc.vector.tensor_copy(out=b16, in_=bstage)

        # ones columns of the v'' layout
        for t in range(NF):
            for sub in range(2):
                v3 = v_sb[:, t, sub, :].rearrange("p (h w) -> p h w", w=65)
                nc.vector.memset(v3[:, :, 64:65], 1.0)
        cv3 = cls_v65[:, :].rearrange("p (h w) -> p h w", w=65)
        nc.vector.memset(cv3[:, :, 64:65], 1.0)

        # ---------------- P1..P3: load weights/x, build xT, qT/kT, v ---------
        # Every dma_start writes a one-shot disjoint region: the hardware
        # dynamic-DGE DMA instruction supports only ONE semaphore wait, and
        # pool-slot reuse makes Tile attach two (reader-release + cross-queue
        # WAW), which walrus rejects ("Too many sync wait commands").
        with tc.tile_pool(name="p13", bufs=1) as p13, \
             tc.tile_pool(name="pp1", bufs=2, space="PSUM") as pp1, \
             tc.tile_pool(name="pp2", bufs=4, space="PSUM") as pp2:

            w16 = p13.tile([128, G, C3], F16)            # W_qkv row-blocks
            xT = p13.tile([128, G, NTOT], F16)           # x^T   [dim, n]

            with tc.tile_pool(name="wstage", bufs=1) as wstage:
                w32 = wstage.tile([128, G, C3], F32)
                for k in range(G):
                    nc.sync.dma_start(
                        out=w32[:, k, :], in_=wqkv_d[128 * k:128 * (k + 1), :]
                    )
                    nc.vector.tensor_copy(out=w16[:, k, :], in_=w32[:, k, :])

            with tc.tile_pool(name="wstage2", bufs=1) as wstage2:
                wo32 = wstage2.tile([128, G, DIM], F32)
                # memset-before-DMA: the pool region is recycled, and the DMA
                # would otherwise inherit one release-wait per prior accessor
                # proc (hardware DMA allows a single sync wait). The memset
                # (DVE, multi-wait capable) absorbs them all.
                nc.vector.memset(wo32, 0.0)
                for k in range(G):
                    nc.sync.dma_start(
                        out=wo32[:, k, :], in_=wout_d[128 * k:128 * (k + 1), :]
                    )
                    nc.vector.tensor_copy(out=wo16[:, k, :], in_=wo32[:, k, :])

            # x -> fp16 -> PE-transpose into xT
            with tc.tile_pool(name="xstage", bufs=1) as xstage:
                x32 = xstage.tile([128, 13, DIM], F32)
                x16 = xstage.tile([128, 13, DIM], F16)
                nc.vector.memset(x32, 0.0)
                for nt in range(13):
                    r0 = 128 * nt
                    m = min(128, NTOT - r0)
                    nc.sync.dma_start(
                        out=x32[:m, nt, :], in_=x_d[r0:r0 + m, :]
                    )
                    nc.vector.tensor_copy(out=x16[:m, nt, :], in_=x32[:m, nt, :])
                    for db in range(G):
                        pt = pp1.tile([128, 128], F16, tag="pt")
                        nc.tensor.transpose(
                            pt[0:128, 0:m],
                            x16[0:m, nt, 128 * db:128 * (db + 1)],
                            idf16[0:m, 0:m],
                        )
                        nc.vector.tensor_copy(
                            out=xT[:, db, r0:r0 + m], in_=pt[0:128, 0:m]
                        )

            # ---- P2: qT / kT projection (c-groups 0..11 of qkv^T) ----
            for ct in range(12):
                dst = qT if ct < G else kT
                gi = ct % G
                for cs, cw in _nchunks(NTOT, 512):
                    pq = pp2.tile([128, 512], F32, tag="mm")
                    for k in range(G):
                        nc.tensor.matmul(
                            pq[:, 0:cw],
                            w16[:, k, 128 * ct:128 * (ct + 1)],
                            xT[:, k, cs:cs + cw],
                            start=(k == 0),
                            stop=(k == G - 1),
                        )
                    nc.vector.tensor_copy(out=dst[:, gi, cs:cs + cw], in_=pq[:, 0:cw])

            # ---- P3: v natural (frame-padded, 65-interleaved) ----
            def v_proj(row0, m, dst_ap):
                # dst_ap: [m, 12, 64]-style destination (65-strided blocks)
                for ch in range(2):
                    pv = pp2.tile([128, 512], F32, tag="mm", name="pv")
                    for k in range(G):
                        nc.tensor.matmul(
                            pv[0:m, 0:384],
                            xT[:, k, row0:row0 + m],
                            w16[:, k, 2 * DIM + 384 * ch:2 * DIM + 384 * (ch + 1)],
                            start=(k == 0),
                            stop=(k == G - 1),
                        )
                    src3 = pv[0:m, 0:384].rearrange("p (h w) -> p h w", w=64)
                    nc.vector.tensor_copy(
                        out=dst_ap[:, 6 * ch:6 * (ch + 1), 0:64], in_=src3
                    )

            v_proj(0, 1, cls_v65[0:1, :].rearrange("p (h w) -> p h w", w=65))
            # frame j-split {96, 100}: the cls key/value rides at row 96 of
            # sub0 (96 is a legal engine partition base; 68 is not)
            for t in range(NF):
                for sub in range(2):
                    m = 96 if sub == 0 else 100
                    row0 = 1 + N * t + 96 * sub
                    dst3 = v_sb[0:m, t, sub, :].rearrange("p (h w) -> p h w", w=65)
                    v_proj(row0, m, dst3)
            for t in range(NF):
                nc.gpsimd.tensor_copy(
                    out=v_sb[96:97, t, 0, :], in_=cls_v65[0:1, :]
                )

        # ---------------- P4: cls-as-query attention row ----------------------
        p45 = ctx.enter_context(tc.tile_pool(name="p45", bufs=1))
        attnT = p45.tile([128, G, NTOT], F16)   # attention out^T
        e12 = p45.tile([H, NTOT], F16)          # exp(cls-query sims)
        a12 = p45.tile([H, NTOT], F32)          # normalized cls attention (output)
        a16 = p45.tile([H, NTOT], F16)
        s_cls = p45.tile([H, 1], F32)
        rs_cls = p45.tile([H, 1], F32)
        # selector weights: cq12[:, g, 2g+p] = cls_q for head 2g+p, else 0 ->
        # the 6 per-group matmuls accumulate into one [12, n] psum
        cq12 = p45.tile([128, G, H], F16)

        with tc.tile_pool(name="pp4", bufs=4, space="PSUM") as pp4:
            nc.vector.memset(cq12, 0.0)
            for g in range(G):
                nc.gpsimd.tensor_copy(
                    out=cq12[0:64, g, 2 * g:2 * g + 1], in_=qT[0:64, g, 0:1]
                )
                nc.gpsimd.tensor_copy(
                    out=cq12[64:128, g, 2 * g + 1:2 * g + 2], in_=qT[64:128, g, 0:1]
                )
            for cs, cw in _nchunks(NTOT, 512):
                pA = pp4.tile([H, 512], F32, tag="pA")
                for g in range(G):
                    nc.tensor.matmul(
                        pA[:, 0:cw], cq12[:, g, :], kT[:, g, cs:cs + cw],
                        start=(g == 0), stop=(g == G - 1),
                    )
                nc.scalar.activation(
                    out=e12[:, cs:cs + cw], in_=pA[:, 0:cw], func=EXPF, scale=SCALE
                )

        nc.vector.reduce_sum(out=s_cls, in_=e12, axis=mybir.AxisListType.X)
        nc.vector.reciprocal(out=rs_cls, in_=s_cls)
        # a12 = e12 * rs (per-partition scale) with f16->f32 cast, on ScalarE
        nc.scalar.activation(
            out=a12, in_=e12, func=mybir.ActivationFunctionType.Copy, scale=rs_cls
        )
        nc.sync.dma_start(out=clsa_d[:, :], in_=a12)
        nc.vector.tensor_copy(out=a16, in_=a12)

        # ---------------- P5: frame attention (96 head x frame tiles) --------
        with tc.tile_pool(name="p5t", bufs=3) as p5t, \
             tc.tile_pool(name="pp5", bufs=2, space="PSUM") as pp5:
            for h in range(H):
                g, po = h // 2, 64 * (h % 2)
                for t in range(NF):
                    qs = 1 + N * t
                    ps = pp5.tile([128, 392], F32, tag="ps")
                    # j-tile0: frame keys 0:96 (+ cls-key sim row at part 96)
                    nc.tensor.matmul(
                        ps[0:96, 0:N],
                        kT[po:po + 64, g, qs:qs + 96],
                        qT[po:po + 64, g, qs:qs + N],
                        start=True, stop=True,
                    )
                    ps_cls = pp5.tile([1, N], F32, tag="ps_cls")
                    nc.tensor.matmul(
                        ps_cls,
                        kT[po:po + 64, g, 0:1],
                        qT[po:po + 64, g, qs:qs + N],
                        start=True, stop=True,
                    )
                    # j-tile1: frame keys 96:196
                    nc.tensor.matmul(
                        ps[0:100, N:2 * N],
                        kT[po:po + 64, g, qs + 96:qs + N],
                        qT[po:po + 64, g, qs:qs + N],
                        start=True, stop=True,
                    )
                    eT0 = p5t.tile([97, N], F16, tag="eT0")
                    eT1 = p5t.tile([100, N], F16, tag="eT1")
                    nc.scalar.activation(
                        out=eT0[0:96, :], in_=ps[0:96, 0:N], func=EXPF, scale=SCALE
                    )
                    nc.scalar.activation(
                        out=eT0[96:97, :], in_=ps_cls, func=EXPF, scale=SCALE
                    )
                    nc.scalar.activation(
                        out=eT1, in_=ps[0:100, N:2 * N], func=EXPF, scale=SCALE
                    )
                    po_t = pp5.tile([65, N], F32, tag="po")
                    nc.tensor.matmul(
                        po_t, v_sb[0:97, t, 0, 65 * h:65 * h + 65], eT0,
                        start=True, stop=False,
                    )
                    nc.tensor.matmul(
                        po_t, v_sb[0:100, t, 1, 65 * h:65 * h + 65], eT1,
                        start=False, stop=True,
                    )
                    # normalize: rs = 1/s ; attnT = out2T * (ones (x) rs)
                    rsf = p5t.tile([1, N], F32, tag="rsf")
                    rhi = p5t.tile([1, N], BF16, tag="rhi")
                    rhi32 = p5t.tile([1, N], F32, tag="rhi32")
                    rlo = p5t.tile([1, N], BF16, tag="rlo")
                    nc.vector.reciprocal(out=rsf, in_=po_t[64:65, :])
                    nc.vector.tensor_copy(out=rhi, in_=rsf)
                    nc.vector.tensor_copy(out=rhi32, in_=rhi)
                    nc.vector.tensor_sub(rlo, rsf, rhi32)
                    pM = pp5.tile([64, N], F32, tag="pM")
                    nc.tensor.matmul(pM, onesbf, rhi, start=True, stop=False)
                    nc.tensor.matmul(pM, onesbf, rlo, start=False, stop=True)
                    o32 = p5t.tile([64, N], F32, tag="o32")
                    nc.vector.tensor_copy(out=o32, in_=po_t[0:64, :])
                    nc.vector.tensor_mul(
                        attnT[po:po + 64, g, qs:qs + N], o32, pM
                    )

        # ---------------- P7: cls-token output -> attnT column 0 -------------
        with tc.tile_pool(name="p7t", bufs=3) as p7t, \
             tc.tile_pool(name="pp7", bufs=2, space="PSUM") as pp7a, \
             tc.tile_pool(name="pp7s", bufs=3, space="PSUM") as pp7s:
            co_ps = [
                pp7a.tile([H, 384], F32, tag="pco", name=f"pco{i}") for i in range(2)
            ]
            # seg list: (transposed a_cls slice, matching v rows)
            segs = []
            pt0 = pp7s.tile([128, H], F16, tag="ptseg")
            nc.tensor.transpose(pt0[0:1, 0:H], a16[0:H, 0:1], idf16[0:H, 0:H])
            a0 = p7t.tile([1, H], F16, tag="aseg")
            nc.vector.tensor_copy(out=a0, in_=pt0[0:1, 0:H])
            segs.append((a0, 1, cls_v65[0:1, :].rearrange("p (h w) -> p h w", w=65)))
            for t in range(NF):
                for sub in range(2):
                    kk = 96 if sub == 0 else 100
                    c0 = 1 + N * t + 96 * sub
                    ptk = pp7s.tile([128, H], F16, tag="ptseg")
                    nc.tensor.transpose(
                        ptk[0:kk, 0:H], a16[0:H, c0:c0 + kk], idf16[0:H, 0:H]
                    )
                    ak = p7t.tile([128, H], F16, tag="aseg")
                    nc.vector.tensor_copy(out=ak[0:kk, :], in_=ptk[0:kk, 0:H])
                    v3 = v_sb[0:kk, t, sub, :].rearrange("p (h w) -> p h w", w=65)
                    segs.append((ak, kk, v3))
            for ch in range(2):
                for i, (aseg, kk, v3) in enumerate(segs):
                    nc.tensor.matmul(
                        co_ps[ch],
                        aseg[0:kk, 0:H],
                        v3[:, 6 * ch:6 * (ch + 1), 0:64],
                        start=(i == 0),
                        stop=(i == len(segs) - 1),
                    )
            for ch in range(2):
                nc.vector.tensor_copy(
                    out=co16[:, 384 * ch:384 * (ch + 1)], in_=co_ps[ch]
                )
            for g in range(G):
                ptc = pp7s.tile([128, H], F16, tag="ptc")
                nc.tensor.transpose(
                    ptc[0:128, 0:H], co16[0:H, 128 * g:128 * (g + 1)],
                    idf16[0:H, 0:H],
                )
                nc.vector.tensor_copy(
                    out=attnT[0:64, g, 0:1], in_=ptc[0:64, 2 * g:2 * g + 1]
                )
                nc.vector.tensor_copy(
                    out=attnT[64:128, g, 0:1], in_=ptc[64:128, 2 * g + 1:2 * g + 2]
                )

        # ---------------- P8: output projection + bias ------------------------
        with tc.tile_pool(name="p8t", bufs=3) as p8t, \
             tc.tile_pool(name="pp8", bufs=4, space="PSUM") as pp8:
            for nt in range(13):
                r0 = 128 * nt
                m = min(128, NTOT - r0)
                osb = p8t.tile([128, DIM], F32, tag="osb")
                for ch in range(2):
                    pf = pp8.tile([128, 384], F32, tag="pf")
                    for g in range(G):
                        nc.tensor.matmul(
                            pf[0:m, :],
                            attnT[:, g, r0:r0 + m],
                            wo16[:, g, 384 * ch:384 * (ch + 1)],
                            start=(g == 0),
                            stop=False,
                        )
                    nc.tensor.matmul(
                        pf[0:m, :],
                        ones16[0:1, 0:m],
                        b16[0:1, 384 * ch:384 * (ch + 1)],
                        start=False,
                        stop=True,
                    )
                    nc.vector.tensor_copy(
                        out=osb[0:m, 384 * ch:384 * (ch + 1)], in_=pf[0:m, :]
                    )
                nc.sync.dma_start(out=out_d[r0:r0 + m, :], in_=osb[0:m, :])


_NC_CACHE = None


def _get_nc():
    global _NC_CACHE
    if _NC_CACHE is None:
        _NC_CACHE = build_bass()
    return _NC_CACHE


LAST_RESULTS = None


def kernel(x, W_qkv, W_out, b_out, f):
    import os

    from concourse.bass_utils import run_bass_kernel_spmd

    global LAST_RESULTS
    x = np.ascontiguousarray(np.asarray(x), dtype=np.float32)
    W_qkv = np.ascontiguousarray(np.asarray(W_qkv), dtype=np.float32)
    W_out = np.ascontiguousarray(np.asarray(W_out), dtype=np.float32)
    b_out = np.ascontiguousarray(np.asarray(b_out), dtype=np.float32)
    assert int(f) == NF and x.shape == (B, NTOT, DIM)

    nc = _get_nc()
    in_maps = [
        {"x": x[b], "W_qkv": W_qkv, "W_out": W_out, "b_out": b_out}
        for b in range(B)
    ]
    res = run_bass_kernel_spmd(
        nc, in_maps, list(range(B)),
        trace=bool(os.environ.get("KBENCH_TRACE")),
    )
    LAST_RESULTS = res
    out = np.stack([res.results[b]["out"] for b in range(B)]).astype(np.float32)
    cls_attn = np.stack(
        [res.results[b]["cls_attn"] for b in range(B)]
    ).reshape(B * H, 1, NTOT).astype(np.float32)
    return out, cls_attn
